# revision 2
# baseline (speedup 1.0000x reference)
"""Trainium2 Bass kernel v2 for batched Clifford (Cl(3,1)) geometric product.

out[n, c] = sum_{i,j} CAYLEY[i, j, c] * a[n, i] * b[n, j]

v2 strategy (vs v1): bf16 end-to-end with XBAR DMA-transpose loads.

- Host pre-casts a, b to bf16. Each super-tile of 16384 tokens is viewed as
  [2048 rows, 128 cols] (8 tokens x 16 blades per row) and DMA-transposed by
  the XBAR straight into SBUF as [128, 2048]: partition p = 16*(token%8) +
  blade, free col = token//8. No TensorEngine transposes, no PSUM
  evacuations for layout.
- Per 4096-token sub-tile: 4 expansion matmuls for a (Ea), 4 for b (Eb)
  [K=32, N=512, fp32 PSUM], one scalar-copy evacuation of pA to bf16 SBUF,
  one [128,2048] vector multiply (sA x pB-in-PSUM), 4 contraction matmuls
  (K4, K=128) into a PSUM tile overlaid on pB's slot, one evacuation.
- Output is stored packed ([nsup, 128, 2048] bf16); the host unscrambles
  token order and upcasts to fp32.

Data parallel over 8 NeuronCores: each core handles 131072 rows.
"""
import sys

sys.path.insert(0, "/opt/trn_rl_repo")

import numpy as np

N_TOTAL = 1048576
N_CORES = 8
ROWS_PER_CORE = N_TOTAL // N_CORES   # 131072
P = 128
F = 512                              # psum cols per sub-tile (one bank, fp32)
SUBS = 4                             # sub-tiles per super-tile
SUP = 8 * F * SUBS                   # 16384 tokens per super-tile
NSUP = ROWS_PER_CORE // SUP          # 8


# ---------------------------------------------------------------------------
# Constant construction: gamma matrices, Phi iso, expansion/contraction mats
# ---------------------------------------------------------------------------
def _build_consts():
    X = np.array([[0.0, 1.0], [1.0, 0.0]])
    Z = np.array([[1.0, 0.0], [0.0, -1.0]])
    E = np.array([[0.0, 1.0], [-1.0, 0.0]])
    I2 = np.eye(2)
    # generators of Cl(3,1): squares +1,+1,+1,-1, pairwise anticommuting
    g = [np.kron(X, I2), np.kron(Z, I2), np.kron(E, E), np.kron(E, X)]
    M = []
    for I in range(16):
        m = np.eye(4)
        for bit in range(4):
            if (I >> bit) & 1:
                m = m @ g[bit]
        M.append(m)
    Phi = np.stack([m.reshape(16) for m in M], axis=1)   # [(r,c), blade]
    PhiInv = Phi.T / 4.0                                 # orthogonal basis

    Ea = np.zeros((32, 128), np.float32)
    Eb = np.zeros((32, 128), np.float32)
    K4 = np.zeros((128, 32), np.float32)
    for v in range(2):
        for r in range(4):
            for k in range(4):
                for c in range(4):
                    col = v * 64 + r * 16 + k * 4 + c
                    for f in range(16):
                        Ea[v * 16 + f, col] = Phi[r * 4 + k, f]
                        Eb[v * 16 + f, col] = Phi[k * 4 + c, f]
                    for cb in range(16):
                        K4[col, v * 16 + cb] = PhiInv[cb, r * 4 + c]
    Ea4 = np.concatenate([Ea] * 4, axis=0).astype(np.float32)
    Eb4 = np.concatenate([Eb] * 4, axis=0).astype(np.float32)
    return Ea4, Eb4, K4.astype(np.float32)


def build_program(rows_per_core=ROWS_PER_CORE, repeats=1, dyn_repeats=None):
    import contextlib

    import concourse.bacc as bacc
    import concourse.mybir as mybir
    from concourse.tile import TileContext

    nsup = rows_per_core // SUP
    assert rows_per_core % SUP == 0
    nc = bacc.Bacc("TRN2", target_bir_lowering=False)
    bf = mybir.dt.bfloat16
    f32 = mybir.dt.float32
    a = nc.dram_tensor("a", [rows_per_core, 16], bf, kind="ExternalInput")
    b = nc.dram_tensor("b", [rows_per_core, 16], bf, kind="ExternalInput")
    cEa = nc.dram_tensor("cEa", [128, 128], bf, kind="ExternalInput")
    cEb = nc.dram_tensor("cEb", [128, 128], bf, kind="ExternalInput")
    cK4 = nc.dram_tensor("cK4", [128, 32], bf, kind="ExternalInput")
    o = nc.dram_tensor("o", [nsup, 128, SUP // 8], bf, kind="ExternalOutput")

    W = SUP // 8                                    # 2048 free cols per super
    av = a.rearrange("(s r w) c -> s r (w c)", r=W, w=8)   # [nsup, 2048, 128]
    bv = b.rearrange("(s r w) c -> s r (w c)", r=W, w=8)

    with TileContext(nc) as tc:
        with tc.tile_pool(name="const", bufs=1) as cpool, \
             tc.tile_pool(name="sb", bufs=2) as sb, \
             tc.tile_pool(name="ps", bufs=1, space="PSUM") as ps:
            tEa = cpool.tile([128, 128], bf)
            tEb = cpool.tile([128, 128], bf)
            tK4 = cpool.tile([128, 32], bf)
            nc.sync.dma_start(tEa[:, :], cEa[:, :])
            nc.sync.dma_start(tEb[:, :], cEb[:, :])
            nc.sync.dma_start(tK4[:, :], cK4[:, :])

            loop_cm = (tc.For_i(0, dyn_repeats, 1) if dyn_repeats
                       else contextlib.nullcontext())
            with loop_cm:
              for _rep in range(repeats):
                for s in range(nsup):
                    saT = sb.tile([P, W], bf, tag="saT", name="saT")
                    sbT = sb.tile([P, W], bf, tag="sbT", name="sbT")
                    nc.sync.dma_start_transpose(saT[:, :], av[s])
                    nc.sync.dma_start_transpose(sbT[:, :], bv[s])
                    sout = sb.tile([P, W], bf, tag="sout", name="sout")
                    for k in range(SUBS):
                        cols = slice(F * k, F * (k + 1))
                        pA = ps.tile([P, 4 * F], f32, tag="pA", name="pA")
                        for j in range(4):
                            nc.tensor.matmul(pA[:, F * j:F * (j + 1)],
                                             tEa[32 * j:32 * (j + 1), :],
                                             saT[32 * j:32 * (j + 1), cols],
                                             start=True, stop=True,
                                             tile_position=(32 * j, 0))
                        sA = sb.tile([P, 4 * F], bf, tag="sA", name="sA")
                        nc.scalar.copy(sA[:, :], pA[:, :])
                        pB = ps.tile([P, 4 * F], f32, tag="pB", name="pB")
                        for j in range(4):
                            nc.tensor.matmul(pB[:, F * j:F * (j + 1)],
                                             tEb[32 * j:32 * (j + 1), :],
                                             sbT[32 * j:32 * (j + 1), cols],
                                             start=True, stop=True,
                                             tile_position=(32 * j, 0))
                        spp = sb.tile([P, 4 * F], bf, tag="spp", name="spp")
                        nc.vector.tensor_mul(spp[:, :], sA[:, :], pB[:, :])
                        # pout shares pB's PSUM slot (tag="pB"): its writes
                        # wait for the mul to drain pB; the next sub's pB
                        # waits for pout's evacuation.
                        pout = ps.tile([P, F], f32, tag="pB", name="pout")
                        for j in range(4):
                            nc.tensor.matmul(pout[32 * j:32 * (j + 1), :],
                                             tK4[:, :],
                                             spp[:, F * j:F * (j + 1)],
                                             start=True, stop=True,
                                             tile_position=(0, 32 * j))
                        nc.scalar.copy(sout[:, cols], pout[:, :])
                    nc.scalar.dma_start(o[s], sout[:, :])

    nc.finalize()
    return nc


_CACHE = {}


def make_in_maps(a16, b16):
    import ml_dtypes
    Ea4, Eb4, K4c = _build_consts()
    md = ml_dtypes.bfloat16
    consts = {"cEa": Ea4.astype(md), "cEb": Eb4.astype(md),
              "cK4": K4c.astype(md)}
    in_maps = []
    for i in range(N_CORES):
        sl = slice(i * ROWS_PER_CORE, (i + 1) * ROWS_PER_CORE)
        in_maps.append({"a": a16[sl], "b": b16[sl], **consts})
    return in_maps


def unpack_output(o_core):
    """[NSUP, 128, W] bf16 packed -> [ROWS_PER_CORE, 16] fp32."""
    o = np.asarray(o_core).astype(np.float32)
    o = o.reshape(NSUP, 8, 16, SUBS, F)        # s, w, c, k, t8
    o = o.transpose(0, 3, 4, 1, 2)             # s, k, t8, w, c
    return np.ascontiguousarray(o.reshape(ROWS_PER_CORE, 16))


def kernel(a, b):
    import ml_dtypes
    from concourse.bass_utils import run_bass_kernel_spmd

    a16 = np.asarray(a, dtype=np.float32).astype(ml_dtypes.bfloat16)
    b16 = np.asarray(b, dtype=np.float32).astype(ml_dtypes.bfloat16)
    assert a16.shape == (N_TOTAL, 16) and b16.shape == (N_TOTAL, 16)
    if "nc" not in _CACHE:
        _CACHE["nc"] = build_program()
    nc = _CACHE["nc"]
    in_maps = make_in_maps(a16, b16)
    res = run_bass_kernel_spmd(nc, in_maps, core_ids=list(range(N_CORES)))
    return np.concatenate([unpack_output(res.results[i]["o"])
                           for i in range(N_CORES)], axis=0)
